# revision 1
# baseline (speedup 1.0000x reference)
"""Cross-attention kernel for Trainium2, SPMD over 8 NeuronCores.

Problem: B=2, LQ=1024, LK=10000, E=256, H=8 heads of D=32.
  q = queries @ Wq + bq ; k = bev @ Wk + bk ; v = bev @ Wv + bv
  out = softmax(q k^T) v  @ Wo + bo

Sharding: core c -> (batch b = c // 4, head-pair hp = c % 4).  Each core
computes attention for its 2 heads of its batch plus the partial output
projection through its 64 rows of Wo.  Host sums the 4 partials per batch
and adds bo (plus the bv @ Wo term, see below).

Key structural choices vs a straightforward implementation:
  - Inputs are transposed on the HOST (pure layout staging, zero flops), so
    the device never runs the expensive PE transposes of xq/xk.  All
    energy/projection matmuls are fp32r, which runs at full PE rate for
    N >= 256 moving columns.
  - bk is dropped entirely: a key-side bias adds a per-query constant to
    every energy in the row, which softmax is invariant to (exact).
  - bv is dropped on device: softmax weights sum to 1, so +bv passes
    through attention unchanged; (bv @ Wo) is added into the host-side
    bias along with bo (exact).
  - The softmax denominator comes from an extra all-ones column appended
    to v, so it falls out of the same PE matmuls that compute attn @ v.
  - exp() is split between the Scalar engine (exact table exp) and the
    Vector engine, which computes a Schraudolph-style exp: bf16 bits are
    round(x * 2^7/ln2 + (127*2^7 - 7.35)) as int16.  The -7.35 debias
    makes the approximation mean-preserving so mixing exact and
    approximate k-tiles does not tilt the softmax average.  Per-element
    noise is ~1.8% rms which averages out over the 10k-key weighted sum.
  - Energy matmuls (K=32) run 4-way row-packed; attn@v matmuls (M=33)
    run 2-way column-packed; k/v projections share one stationary matrix
    [Wk | Wv] so they fill all 128 PE columns.
  - Reciprocal of the 2048 denominators is done in a [128, 16] layout
    (tiny PE transposes in, 0-stride-broadcast matmuls out) instead of
    [1, 512] rows, which would run 30x slower on the per-lane DVE.
"""
import sys

sys.path.insert(0, "/opt/trn_rl_repo")

import numpy as np

B, LQ, LK, E, H = 2, 1024, 10000, 256, 8
D = 32            # head dim
HPC = 2           # heads per core
DC = D * HPC      # 64 projected dims per core
LKP = 10240       # LK padded to a multiple of 512
NKT = LKP // 128  # 80 k-tiles
NCH = LKP // 512  # 20 dma chunks

# Schraudolph exp constants (bf16 bits via int16).  The 7.35 debias makes
# E[approx/exp] = 1 over the energy distribution so exact and approximate
# tiles mix without bias.
SCH_A = float(2.0**7 / np.log(2.0))
SCH_B = float(127.0 * 128.0 - 7.35)

# exp engine split pattern per stg group: True = scalar engine (exact),
# False = vector engine (Schraudolph).
SPLIT = (True, True, False, True, False, True, False, True)

_CACHE = {}


def _build():
    import concourse.bacc as bacc
    import concourse.tile as tile
    from concourse import mybir

    FP32 = mybir.dt.float32
    FP32R = mybir.dt.float32r
    BF16 = mybir.dt.bfloat16
    I16 = mybir.dt.int16
    AF = mybir.ActivationFunctionType
    ALU = mybir.AluOpType

    nc = bacc.Bacc("TRN2", target_bir_lowering=False)

    XQT = nc.dram_tensor("xqt", [128, 2, LQ], FP32R, kind="ExternalInput")
    XKT = nc.dram_tensor("xkt", [128, 2, LKP], FP32R, kind="ExternalInput")
    WQ = nc.dram_tensor("wq", [128, 2, DC], FP32R, kind="ExternalInput")
    WKV = nc.dram_tensor("wkv", [128, 2, 128], FP32R, kind="ExternalInput")
    WO = nc.dram_tensor("wo", [32, 2, E], FP32R, kind="ExternalInput")
    BQ = nc.dram_tensor("bq", [DC], FP32, kind="ExternalInput")
    IDT = nc.dram_tensor("ident", [128, 128], FP32, kind="ExternalInput")
    # partial output, transposed: rows = embed dim, cols = query position
    OUT = nc.dram_tensor("out_t", [E, LQ], FP32, kind="ExternalOutput")

    n_grp = [0]

    with tile.TileContext(nc) as tc:
        with (
            tc.tile_pool(name="singles", bufs=1) as sg,
            tc.tile_pool(name="stt", bufs=4) as stp,
            tc.tile_pool(name="wk", bufs=2) as wkp,
            tc.tile_pool(name="avps", bufs=1, space="PSUM") as avp,
        ):
            # ---- constants / weights ----
            ident = sg.tile([128, 128], FP32, tag="ident")
            nc.sync.dma_start(out=ident, in_=IDT[:, :])
            identr = sg.tile([128, 128], FP32R, tag="identr")
            nc.vector.tensor_copy(identr, ident)
            # random-data fp32 tile for the HAM warm matmuls: the activity
            # monitor tracks actual array toggling, and identity (mostly
            # zeros) barely registers
            wrm = sg.tile([128, 128], FP32, tag="wrm")


            wq_r = sg.tile([128, 2, DC], FP32R, tag="wq")
            nc.sync.dma_start(out=wq_r, in_=WQ[:, :, :])
            wkv_r = sg.tile([128, 2, 128], FP32R, tag="wkv")
            nc.sync.dma_start(out=wkv_r, in_=WKV[:, :, :])
            wo_r = sg.tile([32, 2, E], FP32R, tag="wo")
            nc.sync.dma_start(out=wo_r, in_=WO[:, :, :])
            bq_sb = sg.tile([64, 1], FP32, tag="bq")
            nc.sync.dma_start(out=bq_sb, in_=BQ[:].rearrange("(p o) -> p o", o=1))

            xqT = sg.tile([128, 2, LQ], FP32R, tag="xqT")
            nc.sync.dma_start(out=xqT, in_=XQT[:, :, :])
            nc.vector.tensor_copy(wrm, xqT[:, 0, 0:128])
            xkT = sg.tile([128, 2, LKP], FP32R, tag="xkT")

            # kT/qT rows 0-63 = heads {h0, h1}; rows 64-127 = a copy so the
            # energy matmuls can run 4-way in distinct PE row groups.
            qT = sg.tile([128, LQ], FP32R, tag="qT")
            kT = sg.tile([128, LKP], FP32R, tag="kT")
            v_aug = sg.tile([128, NKT * 66], BF16, tag="vaug")
            # ones columns of v_aug (softmax-denominator trick)
            nc.vector.memset(
                v_aug[:, :].rearrange("p (k t o) -> p k t o", t=2, o=33)[:, :, :, 32:33],
                1.0)

            av = {}
            avs_t = {}

            def warm(st, n):
                # HAM clock-gate feed: only plain-fp32 matmuls register as
                # PE activity.  Flipping the gate warm needs a ~3.4us dense
                # fp32 burst; holding it needs one fp32 matmul inside every
                # ~3.4us window.  These write into stg slots that the next
                # energy matmul overwrites, so they cost no extra PSUM.
                for _ in range(n):
                    nc.tensor.matmul(st[0:32, 0:128], wrm[:, 0:32],
                                     wrm[:, :], start=True, stop=True,
                                     skip_group_check=True)

            def emit_unit(kt, h, qc, st, i):
                # one energy matmul into its stg slot (4-way row-packed)
                row = 32 * h + 64 * (kt % 2)
                qs = slice(qc * 512, (qc + 1) * 512)
                nc.tensor.matmul(
                    st[:, i * 512:(i + 1) * 512],
                    kT[row:row + 32, kt * 128:(kt + 1) * 128],
                    qT[row:row + 32, qs],
                    start=True, stop=True, tile_position=(row, 0))

            pending = []

            def flush_av(depth=2):
                # attn@v accumulations, delayed two groups so the PE
                # (in-order queue) never reaches them before their exp is
                # done: two newer groups' energies run while the exp streams
                # on ACT/DVE.
                if len(pending) < depth:
                    return
                units, sT, qc = pending.pop(0)
                for i, (kt, h) in enumerate(units):
                    off = 64 * h
                    # start=True only on the very first write to this bank:
                    # it clears has_written for the WHOLE bank.  Later
                    # matmuls use start=False: overwrite-where-unset /
                    # accumulate-where-set, which is exactly right for the
                    # h0/h1 regions sharing the bank.
                    nc.tensor.matmul(
                        av[qc][off:off + 33, :],
                        v_aug[:, kt * 66 + 33 * h:kt * 66 + 33 * h + 33],
                        sT[:, i * 512:(i + 1) * 512],
                        start=(kt == 0 and h == 0), stop=(kt == NKT - 1),
                        skip_group_check=True)

            def emit_group(units, qc, pool):
                g = n_grp[0]
                n_grp[0] += 1
                st = pool.tile([128, 1024], FP32, tag="stg", name=f"stg{g}")
                # group 0 carries the big flip-burst (the earlier one decays
                # during the f32r-only prep); later groups just hold.
                warm(st, 30 if g == 0 else (8 if g < 4 else 1))
                for i, (kt, h) in enumerate(units):
                    emit_unit(kt, h, qc, st, i)
                w = 512 * len(units)
                sT = stp.tile([128, 1024], BF16, tag="sT", name=f"sT{g}")
                if SPLIT[g % len(SPLIT)]:
                    nc.scalar.activation(sT[:, 0:w], st[:, 0:w], AF.Exp)
                else:
                    nc.vector.tensor_scalar(
                        out=sT[:, 0:w].bitcast(I16), in0=st[:, 0:w],
                        scalar1=SCH_A, scalar2=SCH_B,
                        op0=ALU.mult, op1=ALU.add)
                flush_av(2)
                pending.append((units, sT, qc))

            def evac(qc):
                avs = wkp.tile([33, 1024], FP32, tag="avs", name=f"avs{qc}")
                for h in range(HPC):
                    nc.vector.tensor_copy(avs[:, h * 512:(h + 1) * 512],
                                          av[qc][64 * h:64 * h + 33, :])
                avs_t[qc] = avs

            # =========== single pass: stream chunks, attention for both
            # query halves per chunk (one av bank per half)
            with (
                tc.tile_pool(name="stg0", bufs=2, space="PSUM") as ps0,
                tc.tile_pool(name="kvp", bufs=1, space="PSUM") as kvp,
                tc.tile_pool(name="vpsp", bufs=1, space="PSUM") as vpsp,
            ):
                # q projection first so energies can start with chunk 0
                # (borrows a stg psum tile)
                stq = ps0.tile([128, 1024], FP32, tag="stg", name="stq")
                warm(stq, 30)
                for qc in range(2):
                    qp = stq[0:64, qc * 512:(qc + 1) * 512]
                    for e in range(2):
                        nc.tensor.matmul(qp, wq_r[:, e, :],
                                         xqT[:, e, qc * 512:(qc + 1) * 512],
                                         start=(e == 0), stop=(e == 1))
                    nc.vector.tensor_scalar_add(
                        qT[0:64, qc * 512:(qc + 1) * 512], qp, bq_sb[:, 0:1])
                nc.sync.dma_start(out=qT[64:128, :], in_=qT[0:64, :])

                av[0] = avp.tile([128, 512], FP32, tag="av_0", name="av_q0")
                av[1] = avp.tile([128, 512], FP32, tag="av_1", name="av_q1")

                for c in range(NCH):
                    cs = slice(c * 512, (c + 1) * 512)
                    nc.sync.dma_start(out=xkT[:, :, cs], in_=XKT[:, :, cs])

                    # k/v projection, one stationary [Wk | Wv]
                    kv = kvp.tile([128, 512], FP32, tag="kv", name=f"kv{c}")
                    for e in range(2):
                        nc.tensor.matmul(kv, wkv_r[:, e, :], xkT[:, e, cs],
                                         start=(e == 0), stop=(e == 1))
                    nc.vector.tensor_copy(kT[0:64, cs], kv[0:64, :])
                    nc.sync.dma_start(out=kT[64:128, cs], in_=kT[0:64, cs])

                    # v^T -> v
                    vt = wkp.tile([64, 512], FP32R, tag="vt", name=f"vt{c}")
                    nc.vector.tensor_copy(vt, kv[64:128, :])
                    vps = vpsp.tile([128, 256], FP32R, tag="vps", name=f"vps{c}")
                    for m in range(4):
                        nc.tensor.transpose(
                            vps[:, m * 64:(m + 1) * 64],
                            vt[:, m * 128:(m + 1) * 128],
                            identr[0:64, 0:64])
                    nc.vector.tensor_copy(
                        v_aug[:, c * 264:(c + 1) * 264].rearrange(
                            "p (k t o) -> p k t o", t=2, o=33)[:, :, :, 0:32],
                        vps[:, :].rearrange("p (k t d) -> p k t d", t=2, d=32))

                    # attention for this chunk's 4 k-tiles, both query halves;
                    # kt-pair-major order keeps 4 distinct PE row groups hot
                    for p0 in (0, 2):
                        for qc in range(2):
                            for kt in (c * 4 + p0, c * 4 + p0 + 1):
                                emit_group([(kt, 0), (kt, 1)], qc, ps0)

                while pending:
                    flush_av(1)
            evac(0)
            evac(1)

            # =========== normalize + output projection ----
            attnT = sg.tile([32, 2, LQ], FP32R, tag="attnT")
            out_sb = [sg.tile([128, LQ], FP32, tag=f"out{e}", name=f"out{e}")
                      for e in range(2)]
            rT = sg.tile([128, 16], FP32, tag="rT")

            with tc.tile_pool(name="scp", bufs=2, space="PSUM") as scp:
                # transpose the evac'd accumulators [33, 128]->[128, 33];
                # column 32 of each transposed tile is the denominator slice
                for qc in range(2):
                    avT = scp.tile([128, 264], FP32, tag="avT",
                                   name=f"avT{qc}")
                    for h in range(HPC):
                        for j in range(4):
                            m = h * 4 + j
                            nc.tensor.transpose(
                                avT[:, m * 33:(m + 1) * 33],
                                avs_t[qc][0:33, h * 512 + j * 128:
                                          h * 512 + (j + 1) * 128],
                                ident[0:33, 0:33])
                    nc.vector.reciprocal(
                        rT[:, qc * 8:(qc + 1) * 8],
                        avT[:, :].rearrange("p (m o) -> p m o", o=33)[:, :, 32])

                for qc in range(2):
                    for h in range(HPC):
                        rb = scp.tile([32, 512], FP32, tag="rb",
                                      name=f"rb{qc}{h}")
                        for j in range(4):
                            idx = (qc * 2 + h) * 4 + j
                            nc.tensor.matmul(
                                rb[:, j * 128:(j + 1) * 128],
                                rT[:, idx:idx + 1].broadcast_to((128, 32)),
                                ident, start=True, stop=True)
                        nc.vector.tensor_mul(
                            attnT[:, h, qc * 512:(qc + 1) * 512],
                            avs_t[qc][0:32, h * 512:(h + 1) * 512], rb)

                for ec in range(2):
                    for qc in range(2):
                        po = scp.tile([128, 512], FP32, tag="po",
                                      name=f"po{ec}{qc}")
                        for h in range(HPC):
                            nc.tensor.matmul(
                                po, wo_r[:, h, ec * 128:(ec + 1) * 128],
                                attnT[:, h, qc * 512:(qc + 1) * 512],
                                start=(h == 0), stop=(h == 1))
                        nc.vector.tensor_copy(
                            out_sb[ec][:, qc * 512:(qc + 1) * 512], po)

            for ec in range(2):
                nc.sync.dma_start(out=OUT[ec * 128:(ec + 1) * 128, :],
                                  in_=out_sb[ec])

    nc.compile()
    return nc


def _get_nc():
    if "nc" not in _CACHE:
        _CACHE["nc"] = _build()
    return _CACHE["nc"]


def kernel(bev_emb, queries, Wq, bq, Wk, bk, Wv, bv, Wo, bo):
    from concourse.bass_utils import run_bass_kernel_spmd

    bev_emb = np.asarray(bev_emb, dtype=np.float32)
    queries = np.asarray(queries, dtype=np.float32)
    Wq = np.asarray(Wq, dtype=np.float32)
    bq = np.asarray(bq, dtype=np.float32)
    Wk = np.asarray(Wk, dtype=np.float32)
    bk = np.asarray(bk, dtype=np.float32)
    Wv = np.asarray(Wv, dtype=np.float32)
    bv = np.asarray(bv, dtype=np.float32)
    Wo = np.asarray(Wo, dtype=np.float32)
    bo = np.asarray(bo, dtype=np.float32)

    ident = np.eye(128, dtype=np.float32)

    # host-side layout staging (no flops): transposes + padding
    xqt = []
    xkt = []
    for b in range(B):
        t = np.ascontiguousarray(
            queries[b].T.reshape(2, 128, LQ).transpose(1, 0, 2))
        xqt.append(t)
        kp = np.zeros((128, 2, LKP), dtype=np.float32)
        kp[:, :, :LK] = bev_emb[b].T.reshape(2, 128, LK).transpose(1, 0, 2)
        xkt.append(kp)

    in_maps = []
    for c in range(8):
        b, hp = c // 4, c % 4
        hs = slice(hp * DC, (hp + 1) * DC)
        wkv = np.concatenate([Wk[:, hs], Wv[:, hs]], axis=1)  # [256, 128]
        in_maps.append({
            "xqt": xqt[b],
            "xkt": xkt[b],
            "wq": np.ascontiguousarray(
                Wq[:, hs].reshape(2, 128, DC).transpose(1, 0, 2)),
            "wkv": np.ascontiguousarray(
                wkv.reshape(2, 128, 128).transpose(1, 0, 2)),
            "wo": np.ascontiguousarray(
                Wo[hs, :].reshape(2, 32, E).transpose(1, 0, 2)),
            "bq": np.ascontiguousarray(bq[hs]),
            "ident": ident,
        })

    nc = _get_nc()
    _CACHE["last_in_maps"] = in_maps
    res = run_bass_kernel_spmd(nc, in_maps, list(range(8)))
    _CACHE["last_result"] = res

    out = np.zeros((B, LQ, E), dtype=np.float32)
    for c in range(8):
        out[c // 4] += res.results[c]["out_t"].T
    # bk drops out of softmax exactly; bv rides through attention into the
    # output projection: out += bv @ Wo.  Both folded into the host bias.
    out += bo + bv @ Wo
    return out



# revision 10
# speedup vs baseline: 1.3070x; 1.3070x over previous
"""Cross-attention kernel for Trainium2, SPMD over 8 NeuronCores.

Problem: B=2, LQ=1024, LK=10000, E=256, H=8 heads of D=32.
  q = queries @ Wq + bq ; k = bev @ Wk + bk ; v = bev @ Wv + bv
  out = softmax(q k^T) v  @ Wo + bo

Sharding: core c -> (batch b = c // 4, head-pair hp = c % 4).  Each core
computes attention for its 2 heads of its batch plus the partial output
projection through its 64 rows of Wo.  Host sums the 4 partials per batch
and adds bo (plus the bv @ Wo term, see below).

Structural choices:
  - All hot-loop matmuls are 16-bit: q/k/v in fp16 (energies need the
    mantissa; fp16 streams 1 col/cycle vs 2 for fp32r), softmax weights in
    bf16 (exp values up to e^30 overflow fp16's range).  16-bit matmuls
    also feed the PE activity monitor, so no fp32 HAM-warm matmuls needed.
  - bk is dropped entirely (softmax is invariant to a per-query constant);
    bv rides through attention (weights sum to 1) and is folded into the
    host-side bias as bv @ Wo.  Both exact.
  - The softmax denominator comes from an all-ones column appended to v,
    so it falls out of the same PE matmuls that compute attn @ v.
  - exp() alternates per (kt, qc, head) unit between the Scalar engine
    (exact table exp) and the Vector engine computing a Schraudolph exp:
    bf16 bits as round(x * 2^7/ln2 + (127*2^7 - 7.35)) int16.  The -7.35
    debias makes the approximation mean-preserving so exact and
    approximate tiles mix without tilting the softmax average.
  - Energy PSUM tiles are per-head single banks in a 5-deep ring, so an
    exp only gates its own bank and the engines never co-idle waiting for
    a 2-bank group to drain.  k/v projection PSUM and the v-transpose
    PSUM share one further bank (temporally disjoint, same pool tag).
  - Energy matmuls (K=32) run as concurrent row-packed pairs (heads at PE
    rows 0-31 / 32-63); attn@v pairs are column-packed (output partitions
    0-32 / 64-96).
  - k/v projections share one stationary [Wk | Wv]; a single fp16
    [128, 512] PSUM evacuation per chunk yields both the k tile and the
    v^T tile, keeping the Vector engine free for exp.
  - Reciprocal of the 2048 denominators is done in a [128, 16] layout
    (tiny [1, 128] PE transposes in, 0-stride-broadcast matmuls out)
    instead of [1, 512] rows, which would run 30x slower on the per-lane
    DVE.
"""
import sys

sys.path.insert(0, "/opt/trn_rl_repo")

import numpy as np
import ml_dtypes

B, LQ, LK, E, H = 2, 1024, 10000, 256, 8
D = 32            # head dim
HPC = 2           # heads per core
DC = D * HPC      # 64 projected dims per core
LKP = 10240       # LK padded to a multiple of 512
NKT = LKP // 128  # 80 k-tiles
NCH = LKP // 512  # 20 dma chunks

# Schraudolph exp constants (bf16 bits via int16).
SCH_A = float(2.0**7 / np.log(2.0))
SCH_B = float(127.0 * 128.0 - 7.35)

_CACHE = {}


def _build():
    import concourse.bacc as bacc
    import concourse.tile as tile
    from concourse import mybir

    F32 = mybir.dt.float32
    F16 = mybir.dt.float16
    BF16 = mybir.dt.bfloat16
    I16 = mybir.dt.int16
    AF = mybir.ActivationFunctionType
    ALU = mybir.AluOpType

    nc = bacc.Bacc("TRN2", target_bir_lowering=False)

    XQT = nc.dram_tensor("xqt", [128, 2, LQ], F16, kind="ExternalInput")
    XKT = nc.dram_tensor("xkt", [128, 2, LKP], F16, kind="ExternalInput")
    WQ = nc.dram_tensor("wq", [128, 2, DC], F16, kind="ExternalInput")
    WKV = nc.dram_tensor("wkv", [128, 2, 128], F16, kind="ExternalInput")
    WO = nc.dram_tensor("wo", [32, 2, E], BF16, kind="ExternalInput")
    BQ = nc.dram_tensor("bq", [DC], F32, kind="ExternalInput")
    IDT = nc.dram_tensor("ident", [128, 128], F32, kind="ExternalInput")
    IDB = nc.dram_tensor("identb", [128, 64], F16, kind="ExternalInput")
    # partial output, transposed: rows = embed dim, cols = query position
    OUT = nc.dram_tensor("out_t", [E, LQ], F32, kind="ExternalOutput")

    with tile.TileContext(nc) as tc:
        with (
            tc.tile_pool(name="singles", bufs=1) as sg,
            tc.tile_pool(name="stt", bufs=8) as stp,
            tc.tile_pool(name="ktp", bufs=2) as ktp,
            tc.tile_pool(name="avps", bufs=1, space="PSUM") as avp,
        ):
            # ---- inputs, ordered by first use (DMAs serialize on Sync) --
            wkv_r = sg.tile([128, 2, 128], F16, tag="wkv")
            nc.sync.dma_start(out=wkv_r, in_=WKV[:, :, :])
            xkT = sg.tile([128, 2, LKP], F16, tag="xkT")
            nc.sync.dma_start(out=xkT[:, :, 0:512], in_=XKT[:, :, 0:512])
            xqT = sg.tile([128, 2, LQ], F16, tag="xqT")
            nc.sync.dma_start(out=xqT, in_=XQT[:, :, :])
            wq_r = sg.tile([128, 2, DC], F16, tag="wq")
            nc.sync.dma_start(out=wq_r, in_=WQ[:, :, :])
            bq_sb = sg.tile([64, 1], F32, tag="bq")
            nc.sync.dma_start(out=bq_sb, in_=BQ[:].rearrange("(p o) -> p o", o=1))
            identb = sg.tile([128, 64], F16, tag="identb")
            nc.sync.dma_start(out=identb, in_=IDB[:, :])
            ident = sg.tile([128, 128], F32, tag="ident")
            nc.sync.dma_start(out=ident, in_=IDT[:, :])
            wo_r = sg.tile([32, 2, E], BF16, tag="wo")
            nc.sync.dma_start(out=wo_r, in_=WO[:, :, :])

            # warm the ACT exp table before the steady loop (~2.7us load)
            dumm = sg.tile([64, 1], BF16, tag="dumm")
            nc.scalar.activation(dumm, bq_sb, AF.Exp)

            qT = sg.tile([64, LQ], F16, tag="qT")
            v_aug = sg.tile([128, NKT * 66], BF16, tag="vaug")
            # ones columns of v_aug (softmax-denominator trick)
            nc.vector.memset(
                v_aug[:, :].rearrange("p (k o) -> p k o", o=33)[:, :, 32:33],
                1.0)
            zz = sg.tile([1, 640], BF16, tag="zz")
            nc.vector.memset(zz, 0.0)

            av = {}
            kts = {}
            pending = []
            n_grp = [0]

            with (
                tc.tile_pool(name="stg0", bufs=5, space="PSUM") as ps0,
                tc.tile_pool(name="kvp", bufs=1, space="PSUM") as kvp,
            ):
                def dma_chunk(c):
                    cs = slice(c * 512, (c + 1) * 512)
                    nc.sync.dma_start(out=xkT[:, :, cs], in_=XKT[:, :, cs])

                def kv_stage(c):
                    # k/v projection, one stationary [Wk | Wv], one fp16
                    # PSUM evacuation for both k (rows 0-63) and v^T
                    cs = slice(c * 512, (c + 1) * 512)
                    kv = kvp.tile([128, 512], F32, tag="kv", name=f"kv{c}")
                    for e in range(2):
                        nc.tensor.matmul(kv, wkv_r[:, e, :], xkT[:, e, cs],
                                         start=(e == 0), stop=(e == 1))
                    kvt = ktp.tile([128, 512], F16, tag="kvt",
                                   name=f"kvt{c}")
                    nc.scalar.copy(kvt, kv)
                    kts[c] = kvt

                def v_stage(c):
                    # v^T -> v via PE transposes (PSUM bank shared with kv
                    # via the pool tag), then strided bf16 copy
                    kvt = kts[c]
                    vpsf = kvp.tile([128, 512], F32, tag="kv",
                                    name=f"vps{c}")
                    vps = vpsf[:, 0:128].bitcast(F16)
                    for m in range(4):
                        nc.tensor.transpose(
                            vps[:, m * 64:(m + 1) * 64],
                            kvt[64:128, m * 128:(m + 1) * 128],
                            identb[64:128, :])
                    nc.vector.tensor_copy(
                        v_aug[:, c * 264:(c + 1) * 264].rearrange(
                            "p (k t o) -> p k t o", t=2, o=33)[:, :, :, 0:32],
                        vps[:, :].rearrange("p (k t d) -> p k t d", t=2, d=32))

                def flush_av(depth):
                    if len(pending) < depth:
                        return
                    kt, qc, sTs = pending.pop(0)
                    for h in range(HPC):
                        # the K=1 zero matmul below initialized the whole
                        # bank, so every accumulation is start=False
                        nc.tensor.matmul(
                            av[qc][64 * h:64 * h + 33, :],
                            v_aug[:, kt * 66 + 33 * h:kt * 66 + 33 * h + 33],
                            sTs[h],
                            start=False, stop=(kt == NKT - 1),
                            skip_group_check=True)

                def emit_group(kt, qc, ktile):
                    g = n_grp[0]
                    n_grp[0] += 1
                    sts = []
                    for h in range(HPC):
                        st = ps0.tile([128, 512], F32, tag="stg",
                                      name=f"stg{g}h{h}")
                        nc.tensor.matmul(
                            st,
                            ktile[32 * h:32 * h + 32,
                                  (kt % 4) * 128:(kt % 4 + 1) * 128],
                            qT[32 * h:32 * h + 32, qc * 512:(qc + 1) * 512],
                            start=True, stop=True)
                        sts.append(st)
                    sTs = []
                    for h in range(HPC):
                        sT = stp.tile([128, 512], BF16, tag="sT",
                                      name=f"sT{g}h{h}")
                        if (kt + qc + h) % 2 == 0:
                            nc.scalar.activation(sT, sts[h], AF.Exp)
                        else:
                            nc.vector.tensor_scalar(
                                out=sT.bitcast(I16), in0=sts[h],
                                scalar1=SCH_A, scalar2=SCH_B,
                                op0=ALU.mult, op1=ALU.add)
                        sTs.append(sT)
                    flush_av(3)
                    pending.append((kt, qc, sTs))

                # ---- prologue ----
                for c in range(1, NCH):
                    dma_chunk(c)

                # q projection (borrows stg psum tiles)
                for qc in range(2):
                    qp = ps0.tile([128, 512], F32, tag="stg",
                                  name=f"stq{qc}")
                    for e in range(2):
                        nc.tensor.matmul(qp[0:64, :], wq_r[:, e, :],
                                         xqT[:, e, qc * 512:(qc + 1) * 512],
                                         start=(e == 0), stop=(e == 1))
                    nc.vector.tensor_scalar_add(
                        qT[:, qc * 512:(qc + 1) * 512], qp[0:64, :],
                        bq_sb[:, 0:1])

                kv_stage(0)
                v_stage(0)
                av[0] = avp.tile([128, 512], F32, tag="av_0", name="av_q0")
                av[1] = avp.tile([128, 512], F32, tag="av_1", name="av_q1")
                for qc in range(2):
                    # zero-fill the whole accumulator bank (K=1 matmul of
                    # zeros) so has_written covers all 128 partitions
                    nc.tensor.matmul(
                        av[qc][0:128, :], zz[0:1, 0:128], zz[0:1, 128:640],
                        start=True, stop=False, skip_group_check=True)

                # ---- steady state: software-pipelined by one chunk ----
                for c in range(NCH):
                    ktile = kts.pop(c)
                    i = 0
                    for j in range(4):
                        for qc in range(2):
                            emit_group(c * 4 + j, qc, ktile)
                            if i == 3 and c + 1 < NCH:
                                kv_stage(c + 1)
                            if i == 5 and c + 1 < NCH:
                                v_stage(c + 1)
                            i += 1
                while pending:
                    flush_av(1)

            # =========== normalize + output projection ----
            attnT = sg.tile([32, 2, LQ], BF16, tag="attnT")
            out_sb = [sg.tile([128, LQ], F32, tag=f"out{e}", name=f"out{e}")
                      for e in range(2)]
            rT = sg.tile([128, 16], F32, tag="rT")
            avs_t = {}

            with tc.tile_pool(name="scp", bufs=2, space="PSUM") as scp:
                # evacuate the [33, 512] accumulator slabs to partition
                # base 0 (rows 0-31 = dims, row 32 = denominator)
                for qc in range(2):
                    avs = sg.tile([33, 1024], F32, tag=f"avs{qc}",
                                  name=f"avs{qc}")
                    for h in range(HPC):
                        if (qc + h) % 2 == 0:
                            nc.scalar.copy(avs[:, h * 512:(h + 1) * 512],
                                           av[qc][64 * h:64 * h + 33, :])
                        else:
                            nc.vector.tensor_copy(
                                avs[:, h * 512:(h + 1) * 512],
                                av[qc][64 * h:64 * h + 33, :])
                    avs_t[qc] = avs

                # transpose just the denominator rows into [128, 16] so
                # the reciprocal runs wide on the DVE
                avT = scp.tile([128, 16], F32, tag="avT", name="avT")
                for qc in range(2):
                    for h in range(HPC):
                        for j in range(4):
                            idx = (qc * 2 + h) * 4 + j
                            nc.tensor.transpose(
                                avT[:, idx:idx + 1],
                                avs_t[qc][32:33, h * 512 + j * 128:
                                          h * 512 + (j + 1) * 128],
                                ident[32:33, 32:33])
                nc.vector.reciprocal(rT, avT)

                for qc in range(2):
                    for h in range(HPC):
                        rb = scp.tile([32, 512], F32, tag="rb",
                                      name=f"rb{qc}{h}")
                        for j in range(4):
                            idx = (qc * 2 + h) * 4 + j
                            nc.tensor.matmul(
                                rb[:, j * 128:(j + 1) * 128],
                                rT[:, idx:idx + 1].broadcast_to((128, 32)),
                                ident, start=True, stop=True)
                        nc.vector.tensor_mul(
                            attnT[:, h, qc * 512:(qc + 1) * 512],
                            avs_t[qc][0:32, h * 512:(h + 1) * 512], rb)

                for ec in range(2):
                    for qc in range(2):
                        po = scp.tile([128, 512], F32, tag="po",
                                      name=f"po{ec}{qc}")
                        for h in range(HPC):
                            nc.tensor.matmul(
                                po, wo_r[:, h, ec * 128:(ec + 1) * 128],
                                attnT[:, h, qc * 512:(qc + 1) * 512],
                                start=(h == 0), stop=(h == 1))
                        if qc == 0:
                            nc.scalar.copy(out_sb[ec][:, 0:512], po)
                        else:
                            nc.vector.tensor_copy(out_sb[ec][:, 512:1024], po)
                    nc.sync.dma_start(out=OUT[ec * 128:(ec + 1) * 128, :],
                                      in_=out_sb[ec])

    nc.compile()
    return nc


def _get_nc():
    if "nc" not in _CACHE:
        _CACHE["nc"] = _build()
    return _CACHE["nc"]


def kernel(bev_emb, queries, Wq, bq, Wk, bk, Wv, bv, Wo, bo):
    from concourse.bass_utils import run_bass_kernel_spmd

    bev_emb = np.asarray(bev_emb, dtype=np.float32)
    queries = np.asarray(queries, dtype=np.float32)
    Wq = np.asarray(Wq, dtype=np.float32)
    bq = np.asarray(bq, dtype=np.float32)
    Wk = np.asarray(Wk, dtype=np.float32)
    bk = np.asarray(bk, dtype=np.float32)
    Wv = np.asarray(Wv, dtype=np.float32)
    bv = np.asarray(bv, dtype=np.float32)
    Wo = np.asarray(Wo, dtype=np.float32)
    bo = np.asarray(bo, dtype=np.float32)

    BF = ml_dtypes.bfloat16
    ident = np.eye(128, dtype=np.float32)
    identb = np.zeros((128, 64), dtype=np.float16)
    identb[64:128] = np.eye(64, dtype=np.float16)

    # host-side layout staging (no flops): transposes + padding + casts
    xqt = []
    xkt = []
    for b in range(B):
        t = np.ascontiguousarray(
            queries[b].T.reshape(2, 128, LQ).transpose(1, 0, 2))
        xqt.append(t.astype(np.float16))
        kp = np.zeros((128, 2, LKP), dtype=np.float16)
        kp[:, :, :LK] = bev_emb[b].T.reshape(2, 128, LK).transpose(
            1, 0, 2).astype(np.float16)
        xkt.append(kp)

    in_maps = []
    for c in range(8):
        b, hp = c // 4, c % 4
        hs = slice(hp * DC, (hp + 1) * DC)
        wkv = np.concatenate([Wk[:, hs], Wv[:, hs]], axis=1)  # [256, 128]
        in_maps.append({
            "xqt": xqt[b],
            "xkt": xkt[b],
            "wq": np.ascontiguousarray(
                Wq[:, hs].reshape(2, 128, DC).transpose(1, 0, 2)).astype(
                    np.float16),
            "wkv": np.ascontiguousarray(
                wkv.reshape(2, 128, 128).transpose(1, 0, 2)).astype(
                    np.float16),
            "wo": np.ascontiguousarray(
                Wo[hs, :].reshape(2, 32, E).transpose(1, 0, 2)).astype(BF),
            "bq": np.ascontiguousarray(bq[hs]),
            "ident": ident,
            "identb": identb,
        })

    nc = _get_nc()
    _CACHE["last_in_maps"] = in_maps
    res = run_bass_kernel_spmd(nc, in_maps, list(range(8)))
    _CACHE["last_result"] = res

    out = np.zeros((B, LQ, E), dtype=np.float32)
    for c in range(8):
        out[c // 4] += res.results[c]["out_t"].T
    # bk drops out of softmax exactly; bv rides through attention into the
    # output projection: out += bv @ Wo.  Both folded into the host bias.
    out += bo + bv @ Wo
    return out


# revision 13
# speedup vs baseline: 1.8172x; 1.3904x over previous
"""Cross-attention kernel for Trainium2, SPMD over 8 NeuronCores.

Problem: B=2, LQ=1024, LK=10000, E=256, H=8 heads of D=32.
  q = queries @ Wq + bq ; k = bev @ Wk + bk ; v = bev @ Wv + bv
  out = softmax(q k^T) v  @ Wo + bo

Sharding: core c -> (batch b = c // 4, head-pair hp = c % 4).  Each core
computes attention for its 2 heads of its batch plus the partial output
projection through its 64 rows of Wo.  Host sums the 4 partials per batch
and adds bo (plus the bv @ Wo term, see below).

Structural choices:
  - All hot-loop matmuls are 16-bit: q/k/v in fp16 (energies need the
    mantissa; fp16 streams 1 col/cycle vs 2 for fp32r), softmax weights in
    bf16 (exp values up to e^30 overflow fp16's range).  16-bit matmuls
    also feed the PE activity monitor, so no fp32 HAM-warm matmuls needed.
  - bk is dropped entirely (softmax is invariant to a per-query constant);
    bv rides through attention (weights sum to 1) and is folded into the
    host-side bias as bv @ Wo.  Both exact.
  - The softmax denominator comes from an all-ones column appended to v,
    so it falls out of the same PE matmuls that compute attn @ v.
  - exp() alternates per (kt, qc, head) unit between the Scalar engine
    (exact table exp) and the Vector engine computing a Schraudolph exp:
    bf16 bits as round(x * 2^7/ln2 + (127*2^7 - 7.35)) int16.  The -7.35
    debias makes the approximation mean-preserving so exact and
    approximate tiles mix without tilting the softmax average.
  - Energy PSUM tiles are per-head single banks in a 5-deep ring, so an
    exp only gates its own bank and the engines never co-idle waiting for
    a 2-bank group to drain.  k/v projection PSUM and the v-transpose
    PSUM share one further bank (temporally disjoint, same pool tag).
  - Energy matmuls (K=32) run as concurrent row-packed pairs (heads at PE
    rows 0-31 / 32-63); attn@v pairs are column-packed (output partitions
    0-32 / 64-96).
  - k/v projections share one stationary [Wk | Wv]; a single fp16
    [128, 512] PSUM evacuation per chunk yields both the k tile and the
    v^T tile, keeping the Vector engine free for exp.
  - Reciprocal of the 2048 denominators is done in a [128, 16] layout
    (tiny [1, 128] PE transposes in, 0-stride-broadcast matmuls out)
    instead of [1, 512] rows, which would run 30x slower on the per-lane
    DVE.
"""
import sys

sys.path.insert(0, "/opt/trn_rl_repo")

import numpy as np
import ml_dtypes

B, LQ, LK, E, H = 2, 1024, 10000, 256, 8
D = 32            # head dim
HPC = 2           # heads per core
DC = D * HPC      # 64 projected dims per core
LKP = 10240       # LK padded to a multiple of 512
NKT = LKP // 128  # 80 k-tiles
NCH = LKP // 512  # 20 dma chunks

# Schraudolph exp constants (bf16 bits via int16).
SCH_A = float(2.0**7 / np.log(2.0))
SCH_B = float(127.0 * 128.0 - 7.35)

_CACHE = {}


def _build():
    import concourse.bacc as bacc
    import concourse.tile as tile
    from concourse import mybir

    F32 = mybir.dt.float32
    F16 = mybir.dt.float16
    BF16 = mybir.dt.bfloat16
    I16 = mybir.dt.int16
    AF = mybir.ActivationFunctionType
    ALU = mybir.AluOpType

    nc = bacc.Bacc("TRN2", target_bir_lowering=False)

    XQT = nc.dram_tensor("xqt", [128, 2, LQ], F16, kind="ExternalInput")
    XKT = nc.dram_tensor("xkt", [128, 2, LKP], F16, kind="ExternalInput")
    WQ = nc.dram_tensor("wq", [128, 2, DC], F16, kind="ExternalInput")
    WKV = nc.dram_tensor("wkv", [128, 2, 128], F16, kind="ExternalInput")
    WO = nc.dram_tensor("wo", [32, 2, E], BF16, kind="ExternalInput")
    BQ = nc.dram_tensor("bq", [DC], F32, kind="ExternalInput")
    IDT = nc.dram_tensor("ident", [128, 128], F32, kind="ExternalInput")
    IDB = nc.dram_tensor("identb", [128, 64], F16, kind="ExternalInput")
    # partial output, transposed: rows = embed dim, cols = query position
    OUT = nc.dram_tensor("out_t", [E, LQ], F32, kind="ExternalOutput")

    with tile.TileContext(nc) as tc:
        with (
            tc.tile_pool(name="singles", bufs=1) as sg,
            tc.tile_pool(name="stt", bufs=8) as stp,
            tc.tile_pool(name="ktp", bufs=2) as ktp,
            tc.tile_pool(name="avps", bufs=1, space="PSUM") as avp,
        ):
            # ---- inputs, ordered by first use (DMAs serialize on Sync) --
            wkv_r = sg.tile([128, 2, 128], F16, tag="wkv")
            nc.sync.dma_start(out=wkv_r, in_=WKV[:, :, :])
            xkT = sg.tile([128, 2, LKP], F16, tag="xkT")
            nc.sync.dma_start(out=xkT[:, :, 0:512], in_=XKT[:, :, 0:512])
            xqT = sg.tile([128, 2, LQ], F16, tag="xqT")
            nc.sync.dma_start(out=xqT, in_=XQT[:, :, :])
            wq_r = sg.tile([128, 2, DC], F16, tag="wq")
            nc.sync.dma_start(out=wq_r, in_=WQ[:, :, :])
            bq_sb = sg.tile([64, 1], F32, tag="bq")
            nc.sync.dma_start(out=bq_sb, in_=BQ[:].rearrange("(p o) -> p o", o=1))
            identb = sg.tile([128, 64], F16, tag="identb")
            nc.sync.dma_start(out=identb, in_=IDB[:, :])
            ident = sg.tile([128, 128], F32, tag="ident")
            nc.sync.dma_start(out=ident, in_=IDT[:, :])
            wo_r = sg.tile([32, 2, E], BF16, tag="wo")
            nc.sync.dma_start(out=wo_r, in_=WO[:, :, :])

            # warm the ACT exp table before the steady loop (~2.7us load)
            dumm = sg.tile([64, 1], BF16, tag="dumm")
            nc.scalar.activation(dumm, bq_sb, AF.Exp)

            # random-data fp32 tile for the HAM warm matmuls: the activity
            # monitor tracks actual fp32-path array toggling; 16-bit
            # matmuls never register, so the gate must be fed explicitly
            wrm = sg.tile([128, 128], F32, tag="wrm")
            nc.vector.tensor_copy(wrm, xqT[:, 0, 0:128])

            qT = sg.tile([64, LQ], F16, tag="qT")
            v_aug = sg.tile([128, NKT * 66], BF16, tag="vaug")
            # ones columns of v_aug (softmax-denominator trick)
            nc.vector.memset(
                v_aug[:, :].rearrange("p (k o) -> p k o", o=33)[:, :, 32:33],
                1.0)
            zz = sg.tile([1, 640], BF16, tag="zz")
            nc.vector.memset(zz, 0.0)

            av = {}
            kts = {}
            pending = []
            n_grp = [0]

            with (
                tc.tile_pool(name="stg0", bufs=5, space="PSUM") as ps0,
                tc.tile_pool(name="kvp", bufs=1, space="PSUM") as kvp,
            ):
                def dma_chunk(c):
                    cs = slice(c * 512, (c + 1) * 512)
                    nc.sync.dma_start(out=xkT[:, :, cs], in_=XKT[:, :, cs])

                def kv_stage(c):
                    # k/v projection, one stationary [Wk | Wv], one fp16
                    # PSUM evacuation for both k (rows 0-63) and v^T
                    cs = slice(c * 512, (c + 1) * 512)
                    kv = kvp.tile([128, 512], F32, tag="kv", name=f"kv{c}")
                    for e in range(2):
                        nc.tensor.matmul(kv, wkv_r[:, e, :], xkT[:, e, cs],
                                         start=(e == 0), stop=(e == 1))
                    kvt = ktp.tile([128, 512], F16, tag="kvt",
                                   name=f"kvt{c}")
                    nc.scalar.copy(kvt, kv)
                    kts[c] = kvt

                def v_stage(c):
                    # v^T -> v via PE transposes (PSUM bank shared with kv
                    # via the pool tag), then strided bf16 copy
                    kvt = kts[c]
                    vpsf = kvp.tile([128, 512], F32, tag="kv",
                                    name=f"vps{c}")
                    vps = vpsf[:, 0:128].bitcast(F16)
                    for m in range(4):
                        nc.tensor.transpose(
                            vps[:, m * 64:(m + 1) * 64],
                            kvt[64:128, m * 128:(m + 1) * 128],
                            identb[64:128, :])
                    nc.vector.tensor_copy(
                        v_aug[:, c * 264:(c + 1) * 264].rearrange(
                            "p (k t o) -> p k t o", t=2, o=33)[:, :, :, 0:32],
                        vps[:, :].rearrange("p (k t d) -> p k t d", t=2, d=32))

                def flush_av(depth):
                    if len(pending) < depth:
                        return
                    kt, qc, sTs = pending.pop(0)
                    for h in range(HPC):
                        # the K=1 zero matmul below initialized the whole
                        # bank, so every accumulation is start=False
                        nc.tensor.matmul(
                            av[qc][64 * h:64 * h + 33, :],
                            v_aug[:, kt * 66 + 33 * h:kt * 66 + 33 * h + 33],
                            sTs[h],
                            start=False, stop=(kt == NKT - 1),
                            skip_group_check=True)

                def warm(st, n):
                    # HAM clock-gate feed: one plain-fp32 matmul inside
                    # every ~3.4us window holds K=8/8.  Writes into an stg
                    # slot the next energy matmul overwrites (start=True
                    # clears the bank), so it costs no extra PSUM.
                    for _ in range(n):
                        nc.tensor.matmul(st[0:32, 0:128], wrm[:, 0:32],
                                         wrm[:, :], start=True, stop=True,
                                         skip_group_check=True)

                def emit_group(kt, qc, ktile):
                    g = n_grp[0]
                    n_grp[0] += 1
                    sts = [ps0.tile([128, 512], F32, tag="stg",
                                    name=f"stg{g}h{h}") for h in range(HPC)]
                    if g % 4 == 0:
                        warm(sts[0], 1)
                    for h in range(HPC):
                        nc.tensor.matmul(
                            sts[h],
                            ktile[32 * h:32 * h + 32,
                                  (kt % 4) * 128:(kt % 4 + 1) * 128],
                            qT[32 * h:32 * h + 32, qc * 512:(qc + 1) * 512],
                            start=True, stop=True)
                    sTs = []
                    for h in range(HPC):
                        sT = stp.tile([128, 512], BF16, tag="sT",
                                      name=f"sT{g}h{h}")
                        if (kt + qc + h) % 2 == 0:
                            nc.scalar.activation(sT, sts[h], AF.Exp)
                        else:
                            nc.vector.tensor_scalar(
                                out=sT.bitcast(I16), in0=sts[h],
                                scalar1=SCH_A, scalar2=SCH_B,
                                op0=ALU.mult, op1=ALU.add)
                        sTs.append(sT)
                    flush_av(3)
                    pending.append((kt, qc, sTs))

                # ---- prologue ----
                for c in range(1, NCH):
                    dma_chunk(c)

                # q projection (borrows stg psum tiles)
                for qc in range(2):
                    qp = ps0.tile([128, 512], F32, tag="stg",
                                  name=f"stq{qc}")
                    for e in range(2):
                        nc.tensor.matmul(qp[0:64, :], wq_r[:, e, :],
                                         xqT[:, e, qc * 512:(qc + 1) * 512],
                                         start=(e == 0), stop=(e == 1))
                    nc.vector.tensor_scalar_add(
                        qT[:, qc * 512:(qc + 1) * 512], qp[0:64, :],
                        bq_sb[:, 0:1])

                kv_stage(0)
                v_stage(0)
                av[0] = avp.tile([128, 512], F32, tag="av_0", name="av_q0")
                av[1] = avp.tile([128, 512], F32, tag="av_1", name="av_q1")
                for qc in range(2):
                    # zero-fill the whole accumulator bank (K=1 matmul of
                    # zeros) so has_written covers all 128 partitions
                    nc.tensor.matmul(
                        av[qc][0:128, :], zz[0:1, 0:128], zz[0:1, 128:640],
                        start=True, stop=False, skip_group_check=True)

                # ~4us dense fp32 burst right before the steady stream to
                # flip the HAM gate to K=8/8 (small 16-bit matmuls can
                # hold the gate but never flip it)
                wb = ps0.tile([128, 512], F32, tag="stg", name="warmb")
                warm(wb, 9)

                # ---- steady state: software-pipelined by one chunk ----
                for c in range(NCH):
                    ktile = kts.pop(c)
                    i = 0
                    for j in range(4):
                        for qc in range(2):
                            emit_group(c * 4 + j, qc, ktile)
                            if i == 3 and c + 1 < NCH:
                                kv_stage(c + 1)
                            if i == 5 and c + 1 < NCH:
                                v_stage(c + 1)
                            i += 1
                while pending:
                    flush_av(1)

            # =========== normalize + output projection ----
            attnT = sg.tile([32, 2, LQ], BF16, tag="attnT")
            out_sb = [sg.tile([128, LQ], F32, tag=f"out{e}", name=f"out{e}")
                      for e in range(2)]
            rT = sg.tile([128, 16], F32, tag="rT")
            avs_t = {}

            with tc.tile_pool(name="scp", bufs=2, space="PSUM") as scp:
                # evacuate the [33, 512] accumulator slabs to partition
                # base 0 (rows 0-31 = dims, row 32 = denominator)
                for qc in range(2):
                    avs = sg.tile([33, 1024], F32, tag=f"avs{qc}",
                                  name=f"avs{qc}")
                    for h in range(HPC):
                        if (qc + h) % 2 == 0:
                            nc.scalar.copy(avs[:, h * 512:(h + 1) * 512],
                                           av[qc][64 * h:64 * h + 33, :])
                        else:
                            nc.vector.tensor_copy(
                                avs[:, h * 512:(h + 1) * 512],
                                av[qc][64 * h:64 * h + 33, :])
                    avs_t[qc] = avs

                # transpose just the denominator rows into [128, 16] so
                # the reciprocal runs wide on the DVE
                avT = scp.tile([128, 16], F32, tag="avT", name="avT")
                for qc in range(2):
                    for h in range(HPC):
                        for j in range(4):
                            idx = (qc * 2 + h) * 4 + j
                            nc.tensor.transpose(
                                avT[:, idx:idx + 1],
                                avs_t[qc][32:33, h * 512 + j * 128:
                                          h * 512 + (j + 1) * 128],
                                ident[32:33, 32:33])
                nc.vector.reciprocal(rT, avT)

                for qc in range(2):
                    for h in range(HPC):
                        rb = scp.tile([32, 512], F32, tag="rb",
                                      name=f"rb{qc}{h}")
                        for j in range(4):
                            idx = (qc * 2 + h) * 4 + j
                            nc.tensor.matmul(
                                rb[:, j * 128:(j + 1) * 128],
                                rT[:, idx:idx + 1].broadcast_to((128, 32)),
                                ident, start=True, stop=True)
                        nc.vector.tensor_mul(
                            attnT[:, h, qc * 512:(qc + 1) * 512],
                            avs_t[qc][0:32, h * 512:(h + 1) * 512], rb)

                for ec in range(2):
                    for qc in range(2):
                        po = scp.tile([128, 512], F32, tag="po",
                                      name=f"po{ec}{qc}")
                        for h in range(HPC):
                            nc.tensor.matmul(
                                po, wo_r[:, h, ec * 128:(ec + 1) * 128],
                                attnT[:, h, qc * 512:(qc + 1) * 512],
                                start=(h == 0), stop=(h == 1))
                        if qc == 0:
                            nc.scalar.copy(out_sb[ec][:, 0:512], po)
                        else:
                            nc.vector.tensor_copy(out_sb[ec][:, 512:1024], po)
                    nc.sync.dma_start(out=OUT[ec * 128:(ec + 1) * 128, :],
                                      in_=out_sb[ec])

    nc.compile()
    return nc


def _get_nc():
    if "nc" not in _CACHE:
        _CACHE["nc"] = _build()
    return _CACHE["nc"]


def kernel(bev_emb, queries, Wq, bq, Wk, bk, Wv, bv, Wo, bo):
    from concourse.bass_utils import run_bass_kernel_spmd

    bev_emb = np.asarray(bev_emb, dtype=np.float32)
    queries = np.asarray(queries, dtype=np.float32)
    Wq = np.asarray(Wq, dtype=np.float32)
    bq = np.asarray(bq, dtype=np.float32)
    Wk = np.asarray(Wk, dtype=np.float32)
    bk = np.asarray(bk, dtype=np.float32)
    Wv = np.asarray(Wv, dtype=np.float32)
    bv = np.asarray(bv, dtype=np.float32)
    Wo = np.asarray(Wo, dtype=np.float32)
    bo = np.asarray(bo, dtype=np.float32)

    BF = ml_dtypes.bfloat16
    ident = np.eye(128, dtype=np.float32)
    identb = np.zeros((128, 64), dtype=np.float16)
    identb[64:128] = np.eye(64, dtype=np.float16)

    # host-side layout staging (no flops): transposes + padding + casts
    xqt = []
    xkt = []
    for b in range(B):
        t = np.ascontiguousarray(
            queries[b].T.reshape(2, 128, LQ).transpose(1, 0, 2))
        xqt.append(t.astype(np.float16))
        kp = np.zeros((128, 2, LKP), dtype=np.float16)
        kp[:, :, :LK] = bev_emb[b].T.reshape(2, 128, LK).transpose(
            1, 0, 2).astype(np.float16)
        xkt.append(kp)

    in_maps = []
    for c in range(8):
        b, hp = c // 4, c % 4
        hs = slice(hp * DC, (hp + 1) * DC)
        wkv = np.concatenate([Wk[:, hs], Wv[:, hs]], axis=1)  # [256, 128]
        in_maps.append({
            "xqt": xqt[b],
            "xkt": xkt[b],
            "wq": np.ascontiguousarray(
                Wq[:, hs].reshape(2, 128, DC).transpose(1, 0, 2)).astype(
                    np.float16),
            "wkv": np.ascontiguousarray(
                wkv.reshape(2, 128, 128).transpose(1, 0, 2)).astype(
                    np.float16),
            "wo": np.ascontiguousarray(
                Wo[hs, :].reshape(2, 32, E).transpose(1, 0, 2)).astype(BF),
            "bq": np.ascontiguousarray(bq[hs]),
            "ident": ident,
            "identb": identb,
        })

    nc = _get_nc()
    _CACHE["last_in_maps"] = in_maps
    res = run_bass_kernel_spmd(nc, in_maps, list(range(8)))
    _CACHE["last_result"] = res

    out = np.zeros((B, LQ, E), dtype=np.float32)
    for c in range(8):
        out[c // 4] += res.results[c]["out_t"].T
    # bk drops out of softmax exactly; bv rides through attention into the
    # output projection: out += bv @ Wo.  Both folded into the host bias.
    out += bo + bv @ Wo
    return out


# revision 16
# speedup vs baseline: 1.8584x; 1.0227x over previous
"""Cross-attention kernel for Trainium2, SPMD over 8 NeuronCores.

Problem: B=2, LQ=1024, LK=10000, E=256, H=8 heads of D=32.
  q = queries @ Wq + bq ; k = bev @ Wk + bk ; v = bev @ Wv + bv
  out = softmax(q k^T) v  @ Wo + bo

Sharding: core c -> (batch b = c // 4, head-pair hp = c % 4).  Each core
computes attention for its 2 heads of its batch plus the partial output
projection through its 64 rows of Wo.  Host sums the 4 partials per batch
and adds bo (plus the bv @ Wo term, see below).

Structural choices:
  - All hot-loop matmuls are 16-bit: q/k/v in fp16 (energies need the
    mantissa; fp16 streams 1 col/cycle vs 2 for fp32r), softmax weights in
    bf16 (exp values up to e^30 overflow fp16's range).  16-bit matmuls
    also feed the PE activity monitor, so no fp32 HAM-warm matmuls needed.
  - bk is dropped entirely (softmax is invariant to a per-query constant);
    bv rides through attention (weights sum to 1) and is folded into the
    host-side bias as bv @ Wo.  Both exact.
  - The softmax denominator comes from an all-ones column appended to v,
    so it falls out of the same PE matmuls that compute attn @ v.
  - exp() alternates per (kt, qc, head) unit between the Scalar engine
    (exact table exp) and the Vector engine computing a Schraudolph exp:
    bf16 bits as round(x * 2^7/ln2 + (127*2^7 - 7.35)) int16.  The -7.35
    debias makes the approximation mean-preserving so exact and
    approximate tiles mix without tilting the softmax average.
  - Energy PSUM tiles are per-head single banks in a 5-deep ring, so an
    exp only gates its own bank and the engines never co-idle waiting for
    a 2-bank group to drain.  k/v projection PSUM and the v-transpose
    PSUM share one further bank (temporally disjoint, same pool tag).
  - Energy matmuls (K=32) run as concurrent row-packed pairs (heads at PE
    rows 0-31 / 32-63); attn@v pairs are column-packed (output partitions
    0-32 / 64-96).
  - k/v projections share one stationary [Wk | Wv]; a single fp16
    [128, 512] PSUM evacuation per chunk yields both the k tile and the
    v^T tile, keeping the Vector engine free for exp.
  - Reciprocal of the 2048 denominators is done in a [128, 16] layout
    (tiny [1, 128] PE transposes in, 0-stride-broadcast matmuls out)
    instead of [1, 512] rows, which would run 30x slower on the per-lane
    DVE.
"""
import sys

sys.path.insert(0, "/opt/trn_rl_repo")

import numpy as np
import ml_dtypes

B, LQ, LK, E, H = 2, 1024, 10000, 256, 8
D = 32            # head dim
HPC = 2           # heads per core
DC = D * HPC      # 64 projected dims per core
LKP = 10240       # LK padded to a multiple of 512
NKT = LKP // 128  # 80 k-tiles
NKT_RUN = 79      # tile 79 is all padding (LK=10000 < 79*128)
NCH = LKP // 512  # 20 dma chunks

# Schraudolph exp constants (bf16 bits via int16).
SCH_A = float(2.0**7 / np.log(2.0))
SCH_B = float(127.0 * 128.0 - 7.35)

_CACHE = {}


def _build():
    import concourse.bacc as bacc
    import concourse.tile as tile
    from concourse import mybir

    F32 = mybir.dt.float32
    F16 = mybir.dt.float16
    BF16 = mybir.dt.bfloat16
    I16 = mybir.dt.int16
    AF = mybir.ActivationFunctionType
    ALU = mybir.AluOpType

    nc = bacc.Bacc("TRN2", target_bir_lowering=False)

    XQT = nc.dram_tensor("xqt", [128, 2, LQ], F16, kind="ExternalInput")
    XKT = nc.dram_tensor("xkt", [128, 2, LKP], F16, kind="ExternalInput")
    WQ = nc.dram_tensor("wq", [128, 2, DC], F16, kind="ExternalInput")
    WKV = nc.dram_tensor("wkv", [128, 2, 128], F16, kind="ExternalInput")
    WO = nc.dram_tensor("wo", [32, 2, E], BF16, kind="ExternalInput")
    BQ = nc.dram_tensor("bq", [DC], F32, kind="ExternalInput")
    IDT = nc.dram_tensor("ident", [128, 128], F32, kind="ExternalInput")
    IDB = nc.dram_tensor("identb", [128, 64], F16, kind="ExternalInput")
    IDBB = nc.dram_tensor("identbb", [128, 128], BF16, kind="ExternalInput")
    # partial output, transposed: rows = embed dim, cols = query position
    OUT = nc.dram_tensor("out_t", [E, LQ], F32, kind="ExternalOutput")

    with tile.TileContext(nc) as tc:
        with (
            tc.tile_pool(name="singles", bufs=1) as sg,
            tc.tile_pool(name="stt", bufs=8) as stp,
            tc.tile_pool(name="ktp", bufs=2) as ktp,
            tc.tile_pool(name="avps", bufs=1, space="PSUM") as avp,
        ):
            # ---- inputs, ordered by first use (DMAs serialize on Sync) --
            wkv_r = sg.tile([128, 2, 128], F16, tag="wkv")
            nc.sync.dma_start(out=wkv_r, in_=WKV[:, :, :])
            xkT = sg.tile([128, 2, LKP], F16, tag="xkT")
            nc.sync.dma_start(out=xkT[:, :, 0:512], in_=XKT[:, :, 0:512])
            xqT = sg.tile([128, 2, LQ], F16, tag="xqT")
            nc.sync.dma_start(out=xqT, in_=XQT[:, :, :])
            wq_r = sg.tile([128, 2, DC], F16, tag="wq")
            nc.sync.dma_start(out=wq_r, in_=WQ[:, :, :])
            bq_sb = sg.tile([64, 1], F32, tag="bq")
            nc.sync.dma_start(out=bq_sb, in_=BQ[:].rearrange("(p o) -> p o", o=1))
            identb = sg.tile([128, 64], F16, tag="identb")
            nc.sync.dma_start(out=identb, in_=IDB[:, :])
            ident = sg.tile([128, 128], F32, tag="ident")
            nc.sync.dma_start(out=ident, in_=IDT[:, :])
            wo_r = sg.tile([32, 2, E], BF16, tag="wo")
            nc.sync.dma_start(out=wo_r, in_=WO[:, :, :])
            identbb = sg.tile([128, 128], BF16, tag="identbb")
            nc.sync.dma_start(out=identbb, in_=IDBB[:, :])

            # warm the ACT exp table before the steady loop (~2.7us load)
            dumm = sg.tile([64, 1], BF16, tag="dumm")
            nc.scalar.activation(dumm, bq_sb, AF.Exp)

            # random-data fp32 tile for the HAM warm matmuls: the activity
            # monitor tracks actual fp32-path array toggling; 16-bit
            # matmuls never register, so the gate must be fed explicitly
            wrm = sg.tile([128, 128], F32, tag="wrm")
            nc.vector.tensor_copy(wrm, xqT[:, 0, 0:128])

            qT = sg.tile([64, LQ], F16, tag="qT")
            v_aug = sg.tile([128, NKT * 66], BF16, tag="vaug")
            # ones columns of v_aug (softmax-denominator trick)
            nc.vector.memset(
                v_aug[:, :].rearrange("p (k o) -> p k o", o=33)[:, :, 32:33],
                1.0)
            zz = sg.tile([1, 640], BF16, tag="zz")
            nc.vector.memset(zz, 0.0)

            av = {}
            kts = {}
            pending = []
            n_grp = [0]

            def warm(st, n):
                # HAM clock-gate feed: one plain-fp32 matmul inside every
                # ~3.4us window holds K=8/8.  Writes into a PSUM slot the
                # next start=True matmul overwrites, so it costs nothing.
                for _ in range(n):
                    nc.tensor.matmul(st[0:32, 0:128], wrm[:, 0:32],
                                     wrm[:, :], start=True, stop=True,
                                     skip_group_check=True)

            with (
                tc.tile_pool(name="stg0", bufs=5, space="PSUM") as ps0,
                tc.tile_pool(name="kvp", bufs=1, space="PSUM") as kvp,
            ):
                # ~4us dense fp32 burst to flip the HAM gate to K=8/8;
                # overlaps the prologue DMA chain (only needs xqT)
                wb = ps0.tile([128, 512], F32, tag="stg", name="warmb")
                warm(wb, 9)

                def dma_chunk(c):
                    cs = slice(c * 512, (c + 1) * 512)
                    nc.sync.dma_start(out=xkT[:, :, cs], in_=XKT[:, :, cs])

                def kv_stage(c):
                    # k/v projection, one stationary [Wk | Wv], one fp16
                    # PSUM evacuation for both k (rows 0-63) and v^T
                    cs = slice(c * 512, (c + 1) * 512)
                    kv = kvp.tile([128, 512], F32, tag="kv", name=f"kv{c}")
                    for e in range(2):
                        nc.tensor.matmul(kv, wkv_r[:, e, :], xkT[:, e, cs],
                                         start=(e == 0), stop=(e == 1))
                    kvt = ktp.tile([128, 512], F16, tag="kvt",
                                   name=f"kvt{c}")
                    nc.scalar.copy(kvt, kv)
                    kts[c] = kvt

                def v_stage(c):
                    # v^T -> v via PE transposes (PSUM bank shared with kv
                    # via the pool tag), then strided bf16 copy
                    kvt = kts[c]
                    vpsf = kvp.tile([128, 512], F32, tag="kv",
                                    name=f"vps{c}")
                    vps = vpsf[:, 0:128].bitcast(F16)
                    for m in range(4):
                        nc.tensor.transpose(
                            vps[:, m * 64:(m + 1) * 64],
                            kvt[64:128, m * 128:(m + 1) * 128],
                            identb[64:128, :])
                    nc.vector.tensor_copy(
                        v_aug[:, c * 264:(c + 1) * 264].rearrange(
                            "p (k t o) -> p k t o", t=2, o=33)[:, :, :, 0:32],
                        vps[:, :].rearrange("p (k t d) -> p k t d", t=2, d=32))

                def flush_av(depth):
                    if len(pending) < depth:
                        return
                    kt, qc, sTs = pending.pop(0)
                    for h in range(HPC):
                        # the K=1 zero matmul below initialized the whole
                        # bank, so every accumulation is start=False
                        nc.tensor.matmul(
                            av[qc][64 * h:64 * h + 33, :],
                            v_aug[:, kt * 66 + 33 * h:kt * 66 + 33 * h + 33],
                            sTs[h],
                            start=False, stop=(kt == NKT_RUN - 1),
                            skip_group_check=True)

                def emit_group(kt, qc, ktile):
                    g = n_grp[0]
                    n_grp[0] += 1
                    sts = [ps0.tile([128, 512], F32, tag="stg",
                                    name=f"stg{g}h{h}") for h in range(HPC)]
                    if g % 4 == 0:
                        warm(sts[0], 1)
                    for h in range(HPC):
                        nc.tensor.matmul(
                            sts[h],
                            ktile[32 * h:32 * h + 32,
                                  (kt % 4) * 128:(kt % 4 + 1) * 128],
                            qT[32 * h:32 * h + 32, qc * 512:(qc + 1) * 512],
                            start=True, stop=True)
                    sTs = []
                    for h in range(HPC):
                        sT = stp.tile([128, 512], BF16, tag="sT",
                                      name=f"sT{g}h{h}")
                        if (kt + qc + h) % 2 == 0:
                            nc.scalar.activation(sT, sts[h], AF.Exp)
                        else:
                            nc.vector.tensor_scalar(
                                out=sT.bitcast(I16), in0=sts[h],
                                scalar1=SCH_A, scalar2=SCH_B,
                                op0=ALU.mult, op1=ALU.add)
                        sTs.append(sT)
                    flush_av(3)
                    pending.append((kt, qc, sTs))

                # ---- prologue ----
                for c in range(1, NCH):
                    dma_chunk(c)

                # q projection (borrows stg psum tiles)
                for qc in range(2):
                    qp = ps0.tile([128, 512], F32, tag="stg",
                                  name=f"stq{qc}")
                    for e in range(2):
                        nc.tensor.matmul(qp[0:64, :], wq_r[:, e, :],
                                         xqT[:, e, qc * 512:(qc + 1) * 512],
                                         start=(e == 0), stop=(e == 1))
                    nc.vector.tensor_scalar_add(
                        qT[:, qc * 512:(qc + 1) * 512], qp[0:64, :],
                        bq_sb[:, 0:1])

                kv_stage(0)
                v_stage(0)
                av[0] = avp.tile([128, 512], F32, tag="av_0", name="av_q0")
                av[1] = avp.tile([128, 512], F32, tag="av_1", name="av_q1")
                for qc in range(2):
                    # zero-fill the whole accumulator bank (K=1 matmul of
                    # zeros) so has_written covers all 128 partitions
                    nc.tensor.matmul(
                        av[qc][0:128, :], zz[0:1, 0:128], zz[0:1, 128:640],
                        start=True, stop=False, skip_group_check=True)

                # ---- steady state: software-pipelined by one chunk ----
                for c in range(NCH):
                    ktile = kts.pop(c)
                    i = 0
                    for j in range(4):
                        if c * 4 + j >= NKT_RUN:
                            break
                        for qc in range(2):
                            emit_group(c * 4 + j, qc, ktile)
                            if i == 3 and c + 1 < NCH:
                                kv_stage(c + 1)
                            if i == 5 and c + 1 < NCH:
                                v_stage(c + 1)
                            i += 1
                while pending:
                    flush_av(1)

            # =========== normalize + output projection ----
            attnT = sg.tile([32, 2, LQ], BF16, tag="attnT")
            out_sb = [sg.tile([128, LQ], F32, tag=f"out{e}", name=f"out{e}")
                      for e in range(2)]
            rT = sg.tile([128, 16], BF16, tag="rT")
            avs_t = {}

            with tc.tile_pool(name="scp", bufs=2, space="PSUM") as scp:
                # evacuate the [33, 512] accumulator slabs to partition
                # base 0 (rows 0-31 = dims, row 32 = denominator)
                for qc in range(2):
                    avs = sg.tile([33, 1024], BF16, tag=f"avs{qc}",
                                  name=f"avs{qc}")
                    for h in range(HPC):
                        if (qc + h) % 2 == 0:
                            nc.scalar.copy(avs[:, h * 512:(h + 1) * 512],
                                           av[qc][64 * h:64 * h + 33, :])
                        else:
                            nc.vector.tensor_copy(
                                avs[:, h * 512:(h + 1) * 512],
                                av[qc][64 * h:64 * h + 33, :])
                    avs_t[qc] = avs

                # transpose just the denominator rows into [128, 16] so
                # the reciprocal runs wide on the DVE
                # bf16 PSUM writes need 4-byte alignment: use even columns
                avT = scp.tile([128, 32], BF16, tag="avT", name="avT")
                for qc in range(2):
                    for h in range(HPC):
                        for j in range(4):
                            idx = (qc * 2 + h) * 4 + j
                            nc.tensor.transpose(
                                avT[:, 2 * idx:2 * idx + 1],
                                avs_t[qc][32:33, h * 512 + j * 128:
                                          h * 512 + (j + 1) * 128],
                                identbb[32:33, 32:33])
                with nc.allow_low_precision(
                        reason="bf16 denominators: 0.4% rel, within budget"):
                    nc.vector.reciprocal(
                        rT, avT.rearrange("p (m o) -> p m o", o=2)[:, :, 0])

                for qc in range(2):
                    for h in range(HPC):
                        rb = scp.tile([32, 512], F32, tag="rb",
                                      name=f"rb{qc}{h}")
                        warm(rb, 1)
                        for j in range(4):
                            idx = (qc * 2 + h) * 4 + j
                            nc.tensor.matmul(
                                rb[:, j * 128:(j + 1) * 128],
                                rT[:, idx:idx + 1].broadcast_to((128, 32)),
                                identbb, start=True, stop=True)
                        nc.vector.tensor_mul(
                            attnT[:, h, qc * 512:(qc + 1) * 512],
                            avs_t[qc][0:32, h * 512:(h + 1) * 512], rb)

                for ec in range(2):
                    for qc in range(2):
                        po = scp.tile([128, 512], F32, tag="po",
                                      name=f"po{ec}{qc}")
                        if qc == 0:
                            warm(po, 1)
                        for h in range(HPC):
                            nc.tensor.matmul(
                                po, wo_r[:, h, ec * 128:(ec + 1) * 128],
                                attnT[:, h, qc * 512:(qc + 1) * 512],
                                start=(h == 0), stop=(h == 1))
                        if qc == 0:
                            nc.scalar.copy(out_sb[ec][:, 0:512], po)
                        else:
                            nc.vector.tensor_copy(out_sb[ec][:, 512:1024], po)
                    nc.sync.dma_start(out=OUT[ec * 128:(ec + 1) * 128, :],
                                      in_=out_sb[ec])

    nc.compile()
    return nc


def _get_nc():
    if "nc" not in _CACHE:
        _CACHE["nc"] = _build()
    return _CACHE["nc"]


def kernel(bev_emb, queries, Wq, bq, Wk, bk, Wv, bv, Wo, bo):
    from concourse.bass_utils import run_bass_kernel_spmd

    bev_emb = np.asarray(bev_emb, dtype=np.float32)
    queries = np.asarray(queries, dtype=np.float32)
    Wq = np.asarray(Wq, dtype=np.float32)
    bq = np.asarray(bq, dtype=np.float32)
    Wk = np.asarray(Wk, dtype=np.float32)
    bk = np.asarray(bk, dtype=np.float32)
    Wv = np.asarray(Wv, dtype=np.float32)
    bv = np.asarray(bv, dtype=np.float32)
    Wo = np.asarray(Wo, dtype=np.float32)
    bo = np.asarray(bo, dtype=np.float32)

    BF = ml_dtypes.bfloat16
    ident = np.eye(128, dtype=np.float32)
    identb = np.zeros((128, 64), dtype=np.float16)
    identb[64:128] = np.eye(64, dtype=np.float16)

    # host-side layout staging (no flops): transposes + padding + casts
    xqt = []
    xkt = []
    for b in range(B):
        t = np.ascontiguousarray(
            queries[b].T.reshape(2, 128, LQ).transpose(1, 0, 2))
        xqt.append(t.astype(np.float16))
        kp = np.zeros((128, 2, LKP), dtype=np.float16)
        kp[:, :, :LK] = bev_emb[b].T.reshape(2, 128, LK).transpose(
            1, 0, 2).astype(np.float16)
        xkt.append(kp)

    in_maps = []
    for c in range(8):
        b, hp = c // 4, c % 4
        hs = slice(hp * DC, (hp + 1) * DC)
        wkv = np.concatenate([Wk[:, hs], Wv[:, hs]], axis=1)  # [256, 128]
        in_maps.append({
            "xqt": xqt[b],
            "xkt": xkt[b],
            "wq": np.ascontiguousarray(
                Wq[:, hs].reshape(2, 128, DC).transpose(1, 0, 2)).astype(
                    np.float16),
            "wkv": np.ascontiguousarray(
                wkv.reshape(2, 128, 128).transpose(1, 0, 2)).astype(
                    np.float16),
            "wo": np.ascontiguousarray(
                Wo[hs, :].reshape(2, 32, E).transpose(1, 0, 2)).astype(BF),
            "bq": np.ascontiguousarray(bq[hs]),
            "ident": ident,
            "identb": identb,
            "identbb": np.eye(128, dtype=BF),
        })

    nc = _get_nc()
    _CACHE["last_in_maps"] = in_maps
    res = run_bass_kernel_spmd(nc, in_maps, list(range(8)))
    _CACHE["last_result"] = res

    out = np.zeros((B, LQ, E), dtype=np.float32)
    for c in range(8):
        out[c // 4] += res.results[c]["out_t"].T
    # bk drops out of softmax exactly; bv rides through attention into the
    # output projection: out += bv @ Wo.  Both folded into the host bias.
    out += bo + bv @ Wo
    return out
